# revision 1
# baseline (speedup 1.0000x reference)
"""DiffusionAttentionPairBias kernel for Trainium2 (8 NeuronCores, SPMD), v4.

Problem (B=1, N=1024, D_A=768, D_S=384, D_Z=128, H=16, DH=48):
  q_in = sigmoid(LN(s) @ gw_ad + gb_ad) * LN(a) + LN(s) @ bw_ad
  q,k,v,g = projections of q_in;  bias = (LN(z)*zn_g + zn_b) @ zp_w
  attn = softmax(q k^T / sqrt(DH) + bias);  out = sigmoid(g) * (attn v)
  y = sigmoid(s @ sg_w + sg_b) * (out @ ow)

Sharding: pure data-parallel on the query axis (core c owns q rows
[128c, 128c+128)). No collectives; host concatenates.

Structure (driven by TimelineSim cost analysis):
 - All weights bf16 host-side; SCALE and qb folded into 64-stride-padded
   Q/K weights (so Q/K project in natural layout, then one wide xbar
   transpose each into the head-pair layout).
 - z LN mean-correction folds into column-centered projection weights:
     Wc = zn_g*zp_w - colsum(zn_g*zp_w)/DZ
   so bias = rstd * (z @ Wc) exactly (zn_b and the mean term are
   constant along the softmax axis / linear in z respectively).
 - z is DMA'd straight from DRAM through the xbar transpose (ch to
   partitions) - no separate load+transpose, half the DMA bus traffic.
 - Per 16-key tile j (64 iterations): one DVE/ACT square, 4 wide
   matmuls (1024 cols) into a [64, 1024] PSUM tile: PE quadrant 0
   streams zT against [Wc | ones | 0] (P0c rows 0-15, S1 row 16),
   quadrant 1 streams z2T against [ones | 0] (S2 row 32). One [48,1024]
   bf16 evac + one xbar transpose per j returns everything to
   q-partition layout; rstd = 1/sqrt((S2-S1^2/DZ)/DZ+eps) and a single
   multiply writes bias_h.
 - ACT function-table thrash avoided: LN stats for all 19 tiles batched
   into one Sqrt phase, then one Sigmoid phase (square/copy are in
   every ACT table set, so z-loop squares on ACT are free).
 - Scores in PSUM: QK^T matmul + identity-matmul bias fold. Softmax:
   exp without max-subtract (|logits| < 2 here), sum/reciprocal folded
   into the AV epilogue.
"""

import math
import os

import ml_dtypes
import numpy as np

import concourse.bass as bass
import concourse.bacc as bacc
import concourse.mybir as mybir
import concourse.tile as tile
from concourse.masks import make_identity

F32 = mybir.dt.float32
BF16 = mybir.dt.bfloat16
AF = mybir.ActivationFunctionType
ALU = mybir.AluOpType
AX = mybir.AxisListType

N = 1024
DA = 768
DS = 384
DZ = 128
H = 16
DH = 48
HD = 768
HDP = 1024        # padded head dim stride (heads at 64-stride)
QP = 128          # query rows per core
NCORES = 8
SCALE = 1.0 / math.sqrt(DH)
EPS = 1e-5
KJ = 16           # kp rows per z macro-tile
NJ = N // KJ      # 64 z macro-tiles
RW = 32           # PE quadrant row stride


def _flag(name):
    return bool(int(os.environ.get(name, "0")))


def _ap(t, offset_elems, dims):
    """Build an AP on tile `t`: partition dim is inherited from the tile,
    `dims` are explicit free dims [[stride, count], ...]."""
    a0 = t[:]
    return bass.AP(
        tensor=a0.tensor,
        offset=a0.offset + offset_elems,
        ap=[list(a0.ap[0])] + [list(d) for d in dims],
    )


def build_program():
    nc = bacc.Bacc("TRN2", target_bir_lowering=False, debug=False)

    def din(name, shape, dt=BF16):
        return nc.dram_tensor(name, shape, dt, kind="ExternalInput")

    a_d = din("a", [N, DA], F32)
    s_d = din("s", [N, DS], F32)
    aq_d = din("a_q", [QP, DA], F32)
    sq_d = din("s_q", [QP, DS], F32)
    z_d = din("z_q", [QP, N, DZ])
    gwad_d = din("adaln_gw", [DS, DA])
    bwad_d = din("adaln_bw", [DS, DA])
    gbad_d = din("adaln_gb", [DA])
    qwp_d = din("qwp", [DA, HDP])
    qbp_d = din("qbp", [HDP])
    kwp_d = din("kwp", [DA, HDP])
    vw_d = din("vw", [DA, HD])
    gw_d = din("gw", [DA, HD])
    ow_d = din("ow", [HD, DA])
    zng_d = din("zn_g", [DZ], F32)
    zpw_d = din("zp_w", [DZ, H], F32)
    sgw_d = din("sg_w", [DS, DA])
    sgb_d = din("sg_b", [DA])
    out_d = nc.dram_tensor("out", [QP, DA], F32, kind="ExternalOutput")

    with tile.TileContext(nc) as tc:
        with (
            tc.tile_pool(name="const", bufs=1) as cp,
            tc.tile_pool(name="persist", bufs=1) as pp,
            tc.tile_pool(name="wpool", bufs=2) as wp,
            tc.tile_pool(name="act", bufs=2) as ap_,
            tc.tile_pool(name="ztr", bufs=2) as zt_,
            tc.tile_pool(name="zsq", bufs=4) as zq_,
            tc.tile_pool(name="zev", bufs=2) as ze_,
            tc.tile_pool(name="ztt", bufs=2) as ztt,
            tc.tile_pool(name="corr", bufs=2) as co_,
            tc.tile_pool(name="attnp", bufs=2) as atp,
            tc.tile_pool(name="ps_misc", bufs=1, space="PSUM") as ps_m,
            tc.tile_pool(name="ps_z", bufs=3, space="PSUM") as ps_z,
            tc.tile_pool(name="ps_s", bufs=1, space="PSUM") as ps_s,
        ):
            # ---------------- constants / small prep ----------------
            ones_r = cp.tile([1, 128], BF16)
            nc.vector.memset(ones_r, 1.0)
            epsA = cp.tile([128, 1], F32)
            nc.vector.memset(epsA, EPS)
            ident = cp.tile([128, 128], BF16)
            make_identity(nc, ident)
            ones1 = cp.tile([DZ, 1], BF16)
            nc.vector.memset(ones1, 1.0)

            # waugC = [zn_g*zp_w - colsum/DZ | ones | 0-pad]  ([DZ, RW])
            # onesW = [ones | 0-pad]                          ([DZ, RW])
            zng_c = cp.tile([DZ, 1], F32)
            nc.sync.dma_start(out=zng_c, in_=zng_d[:].rearrange("(p o) -> p o", o=1))
            zpw_f = cp.tile([DZ, H], F32)
            nc.sync.dma_start(out=zpw_f, in_=zpw_d[:])
            wprod = cp.tile([DZ, H], F32)
            nc.vector.tensor_scalar_mul(wprod, in0=zpw_f[:], scalar1=zng_c[:])
            wprod_b = cp.tile([DZ, H], BF16)
            nc.vector.tensor_copy(wprod_b, wprod[:])
            ps_cs = ps_m.tile([1, H], F32, tag="misc")
            nc.tensor.matmul(ps_cs, ones1[:], wprod_b[:], start=True, stop=True)
            cs_row = cp.tile([1, H], F32)
            nc.scalar.activation(cs_row, ps_cs[0:1, :], AF.Copy, scale=1.0 / DZ)
            cs_bf = cp.tile([1, H], BF16)
            nc.vector.tensor_copy(cs_bf, cs_row[:])
            ps_csb = ps_m.tile([128, H], F32, tag="misc")
            nc.tensor.matmul(ps_csb, ones_r[:], cs_bf[:], start=True, stop=True)
            waugC = cp.tile([DZ, RW], BF16)
            nc.vector.memset(waugC, 0.0)
            nc.vector.tensor_sub(waugC[:, 0:H], wprod[:], ps_csb[:])
            nc.vector.memset(waugC[:, H : H + 1], 1.0)
            onesW = cp.tile([DZ, RW], BF16)
            nc.vector.memset(onesW, 0.0)
            nc.vector.memset(onesW[:, 0:1], 1.0)

            # per-feature bias rows (bf16, added via K=1 rank-1 matmuls)
            gbad_r = cp.tile([1, DA], BF16)
            nc.gpsimd.dma_start(out=gbad_r, in_=gbad_d[:].rearrange("(o n) -> o n", o=1))
            qbp_r = cp.tile([1, HDP], BF16)
            nc.gpsimd.dma_start(out=qbp_r, in_=qbp_d[:].rearrange("(o n) -> o n", o=1))
            sgb_r = cp.tile([1, DA], BF16)
            nc.gpsimd.dma_start(out=sgb_r, in_=sgb_d[:].rearrange("(o n) -> o n", o=1))

            # ---------------- persistent activations ----------------
            s_lnT = pp.tile([128, 3, N], BF16)
            q_inT = pp.tile([128, 6, N], BF16)
            qi_qT = pp.tile([128, 6, QP], BF16)
            kT = pp.tile([128, 8, N], BF16)          # K^T head-pairs at part 0/64
            qT = pp.tile([128, 8, QP], BF16)         # (Q*SCALE+qb)^T head-pairs
            v_sb = pp.tile([128, 8, HD], BF16)
            sig_g = pp.tile([128, HD], BF16)
            sig_o = pp.tile([128, DA], BF16)
            out_nat = pp.tile([128, HD], BF16)
            bias_h = [
                pp.tile([128, 512, H], BF16, tag=f"bias{i}", name=f"bias{i}")
                for i in range(2)
            ]

            z_off = _flag("Z_OFF")
            attn_off = _flag("ATTN_OFF")

            # ---- z prefetch: stream z in ALREADY TRANSPOSED (DRAM xbar),
            # two j-tiles per DMA ----
            PREF = 2
            zld_tiles = {}

            def z_load(m):
                t = zt_.tile([128, 2 * KJ, DZ], BF16, tag="zT")
                nc.sync.dma_start(
                    out=t,
                    in_=z_d[:, 2 * KJ * m : 2 * KJ * (m + 1), :].rearrange(
                        "p a b -> p (a b)"
                    ),
                    transpose=True,
                )
                zld_tiles[m] = t

            if not z_off:
                for m in range(PREF):
                    z_load(m)

            # ---------------- front ----------------
            # s tiles: single-pass LN (Sqrt table stays resident; no reload)
            def ln_full(src_ap, out_bf):
                xt = ap_.tile([128, DS], F32, tag="lnins")
                nc.gpsimd.dma_start(out=xt, in_=src_ap)
                st6 = ap_.tile([128, 2, 6], F32, tag="lnst")
                nc.vector.bn_stats(out=st6[:, 0, :], in_=xt[:, 0 : DS // 2])
                nc.vector.bn_stats(out=st6[:, 1, :], in_=xt[:, DS // 2 : DS])
                mv = ap_.tile([128, 2], F32, tag="lnmv")
                nc.vector.bn_aggr(out=mv, in_=st6[:, :, :])
                sd = ap_.tile([128, 1], F32, tag="lnsd")
                nc.scalar.activation(sd, mv[:, 1:2], AF.Sqrt, bias=epsA[:])
                rs = ap_.tile([128, 1], F32, tag="lnrs")
                nc.vector.reciprocal(rs, sd[:])
                nc.vector.tensor_scalar(
                    out=out_bf, in0=xt[:], scalar1=mv[:, 0:1], scalar2=rs[:],
                    op0=ALU.subtract, op1=ALU.mult,
                )

            s_srcs = [s_d[t * 128 : (t + 1) * 128, :] for t in range(8)] + [sq_d[:]]
            a_srcs = [a_d[t * 128 : (t + 1) * 128, :] for t in range(8)] + [aq_d[:]]
            for t in range(8):
                s_ln = ap_.tile([128, DS], BF16, tag="sln")
                ln_full(s_srcs[t], s_ln[:])
                nc.sync.dma_start(
                    out=s_lnT[:, :, t * 128 : (t + 1) * 128], in_=s_ln[:], transpose=True
                )
            sq_ln = pp.tile([128, DS], BF16)
            ln_full(sq_d[:], sq_ln[:])
            sq_lnT = pp.tile([128, 3, QP], BF16)
            nc.sync.dma_start(out=sq_lnT[:, :, :], in_=sq_ln[:], transpose=True)

            # a tiles: stats pass now, normalize inside adaln (reload) so the
            # sigmoid phase never interleaves with ACT Sqrt ops
            mv_all = pp.tile([128, 9, 2], F32)
            rs_all = pp.tile([128, 9], F32)
            for t in range(9):
                xt = ap_.tile([128, DA], F32, tag="lnina")
                nc.gpsimd.dma_start(out=xt, in_=a_srcs[t])
                st6 = ap_.tile([128, 2, 6], F32, tag="lnst")
                nc.vector.bn_stats(out=st6[:, 0, :], in_=xt[:, 0 : DA // 2])
                nc.vector.bn_stats(out=st6[:, 1, :], in_=xt[:, DA // 2 : DA])
                nc.vector.bn_aggr(out=mv_all[:, t, :], in_=st6[:, :, :])
            sd_all = pp.tile([128, 9], F32)
            nc.scalar.activation(sd_all, _ap(mv_all, 1, [[2, 9]]), AF.Sqrt, bias=epsA[:])
            nc.vector.reciprocal(rs_all, sd_all[:])

            def ln_norm(t, src_ap, out_bf):
                xt = ap_.tile([128, DA], F32, tag="lnina")
                nc.gpsimd.dma_start(out=xt, in_=src_ap)
                nc.vector.tensor_scalar(
                    out=out_bf, in0=xt[:], scalar1=mv_all[:, t, 0:1],
                    scalar2=rs_all[:, t : t + 1], op0=ALU.subtract, op1=ALU.mult,
                )

            # ---- phase B2: adaln -> q_in (sigmoid only on ACT) ----
            gwad_s = wp.tile([128, 3, DA], BF16, tag="w9")
            nc.gpsimd.dma_start(out=gwad_s, in_=gwad_d[:].rearrange("(t p) n -> p t n", p=128))
            bwad_s = wp.tile([128, 3, DA], BF16, tag="w9")
            nc.gpsimd.dma_start(out=bwad_s, in_=bwad_d[:].rearrange("(t p) n -> p t n", p=128))

            chunks = [(0, 512), (512, 256)]

            def adaln_qin(t, lnT_ap, a_src, out_T, outT_col0):
                a_ln = ap_.tile([128, DA], BF16, tag="aln")
                ln_norm(t, a_src, a_ln[:])
                q_in = ap_.tile([128, DA], BF16, tag="qin")
                for c0, cn in chunks:
                    psG = ps_m.tile([128, cn], F32, tag="misc")
                    for kt in range(3):
                        nc.tensor.matmul(
                            psG,
                            lnT_ap(kt),
                            gwad_s[:, kt, c0 : c0 + cn],
                            start=(kt == 0),
                            stop=False,
                        )
                    nc.tensor.matmul(
                        psG, ones_r[:], gbad_r[:, c0 : c0 + cn], start=False, stop=True
                    )
                    sgG = ap_.tile([128, cn], BF16, tag="sgG")
                    nc.scalar.activation(sgG, psG[:], AF.Sigmoid)
                    psB = ps_m.tile([128, cn], F32, tag="misc")
                    for kt in range(3):
                        nc.tensor.matmul(
                            psB,
                            lnT_ap(kt),
                            bwad_s[:, kt, c0 : c0 + cn],
                            start=(kt == 0),
                            stop=(kt == 2),
                        )
                    tmp = ap_.tile([128, cn], BF16, tag="qtmp")
                    nc.vector.tensor_mul(tmp, sgG[:], a_ln[:, c0 : c0 + cn])
                    nc.vector.tensor_add(q_in[:, c0 : c0 + cn], tmp[:], psB[:])
                nc.sync.dma_start(
                    out=out_T[:, :, outT_col0 : outT_col0 + 128], in_=q_in[:], transpose=True
                )

            for t in range(8):
                adaln_qin(
                    t,
                    lambda kt, t=t: s_lnT[:, kt, t * 128 : (t + 1) * 128],
                    a_srcs[t],
                    q_inT,
                    t * 128,
                )
            adaln_qin(8, lambda kt: sq_lnT[:, kt, :], aq_d[:], qi_qT, 0)

            # ---- K natural through padded weights, then wide transposes ----
            kwp_s = wp.tile([128, 6, HDP], BF16, tag="w9")
            nc.gpsimd.dma_start(out=kwp_s, in_=kwp_d[:].rearrange("(t p) n -> p t n", p=128))
            for t in range(8):
                k_nat = ap_.tile([128, HDP], BF16, tag="knat")
                for c0 in (0, 512):
                    psK = ps_m.tile([128, 512], F32, tag="misc")
                    for kt in range(6):
                        nc.tensor.matmul(
                            psK,
                            q_inT[:, kt, t * 128 : (t + 1) * 128],
                            kwp_s[:, kt, c0 : c0 + 512],
                            start=(kt == 0),
                            stop=(kt == 5),
                        )
                    nc.vector.tensor_copy(k_nat[:, c0 : c0 + 512], psK[:])
                nc.sync.dma_start(
                    out=kT[:, :, t * 128 : (t + 1) * 128], in_=k_nat[:], transpose=True
                )

            # ---- V natural ----
            vw_s = wp.tile([128, 6, HD], BF16, tag="w9")
            nc.gpsimd.dma_start(out=vw_s, in_=vw_d[:].rearrange("(t p) n -> p t n", p=128))
            for t in range(8):
                for c0, cn in chunks:
                    psV = ps_m.tile([128, 512], F32, tag="misc")
                    for kt in range(6):
                        nc.tensor.matmul(
                            psV[:, 0:cn],
                            q_inT[:, kt, t * 128 : (t + 1) * 128],
                            vw_s[:, kt, c0 : c0 + cn],
                            start=(kt == 0),
                            stop=(kt == 5),
                        )
                    nc.vector.tensor_copy(v_sb[:, t, c0 : c0 + cn], psV[:, 0:cn])

            # ---- Q natural through padded+scaled weights ----
            qwp_s = wp.tile([128, 6, HDP], BF16, tag="w9")
            nc.gpsimd.dma_start(out=qwp_s, in_=qwp_d[:].rearrange("(t p) n -> p t n", p=128))
            q_nat = ap_.tile([128, HDP], BF16, tag="knat")
            for c0 in (0, 512):
                psQ = ps_m.tile([128, 512], F32, tag="misc")
                for kt in range(6):
                    nc.tensor.matmul(
                        psQ,
                        qi_qT[:, kt, :],
                        qwp_s[:, kt, c0 : c0 + 512],
                        start=(kt == 0),
                        stop=False,
                    )
                nc.tensor.matmul(
                    psQ, ones_r[:], qbp_r[:, c0 : c0 + 512], start=False, stop=True
                )
                nc.vector.tensor_copy(q_nat[:, c0 : c0 + 512], psQ[:])
            nc.sync.dma_start(out=qT[:, :, :], in_=q_nat[:], transpose=True)

            # ---- G gate ----
            gw_s = wp.tile([128, 6, HD], BF16, tag="w9")
            nc.gpsimd.dma_start(out=gw_s, in_=gw_d[:].rearrange("(t p) n -> p t n", p=128))
            for c0, cn in chunks:
                psg = ps_m.tile([128, cn], F32, tag="misc")
                for kt in range(6):
                    nc.tensor.matmul(
                        psg,
                        qi_qT[:, kt, :],
                        gw_s[:, kt, c0 : c0 + cn],
                        start=(kt == 0),
                        stop=(kt == 5),
                    )
                nc.scalar.activation(sig_g[:, c0 : c0 + cn], psg[:], AF.Sigmoid)

            # ---- output gate from raw s_q ----
            sgw_s = wp.tile([128, 3, DA], BF16, tag="w9")
            nc.gpsimd.dma_start(out=sgw_s, in_=sgw_d[:].rearrange("(t p) n -> p t n", p=128))
            sq_bf = ap_.tile([128, DS], BF16, tag="sqbf")
            nc.gpsimd.dma_start(out=sq_bf, in_=sq_d[:])
            sqT = pp.tile([128, 3, QP], BF16)
            nc.sync.dma_start(out=sqT[:, :, :], in_=sq_bf[:], transpose=True)
            for c0, cn in chunks:
                pso = ps_m.tile([128, cn], F32, tag="misc")
                for kt in range(3):
                    nc.tensor.matmul(
                        pso,
                        sqT[:, kt, :],
                        sgw_s[:, kt, c0 : c0 + cn],
                        start=(kt == 0),
                        stop=False,
                    )
                nc.tensor.matmul(
                    pso, ones_r[:], sgb_r[:, c0 : c0 + cn], start=False, stop=True
                )
                nc.scalar.activation(sig_o[:, c0 : c0 + cn], pso[:], AF.Sigmoid)

            # ---------------- z pipeline ----------------
            if z_off:
                nc.vector.memset(bias_h[0][:], 0.0)
                nc.vector.memset(bias_h[1][:], 0.0)
            else:
                ez = None
                for j in range(NJ):
                    m, jj = j // 2, j % 2
                    if jj == 0:
                        if m + PREF < NJ // 2:
                            z_load(m + PREF)
                        ez = ze_.tile([112, 2 * 8 * DZ], BF16, tag="ez")
                    zT2 = zld_tiles[m]
                    # square (for sum-of-squares); ACT every 4th (square is
                    # in every ACT table set -> no table reload)
                    z2T = zq_.tile([128, KJ, DZ], BF16, tag="z2")
                    zTj = zT2[:, KJ * jj : KJ * (jj + 1), :]
                    if j % 4 == 3:
                        nc.scalar.activation(z2T, zTj, AF.Square)
                    else:
                        nc.vector.tensor_mul(z2T, zTj, zTj)
                    if jj == 1:
                        zld_tiles.pop(m)
                    # one [128, 1024] psum per j, all 4 PE quadrants:
                    #  q0: P0c+S1 kp 0-8 (zT vs waugC)   q1: S2 kp 0-8 (z2T vs onesW)
                    #  q2: P0c+S1 kp 8-16                q3: S2 kp 8-16
                    pz = ps_z.tile([128, 8 * DZ], F32, tag="pz")
                    for g in range(2):
                        for c in range(2):
                            nc.tensor.matmul(
                                pz[2 * g * RW : (2 * g + 1) * RW, 512 * c : 512 * (c + 1)],
                                waugC[:],
                                zT2[
                                    :,
                                    KJ * jj + 8 * g + 4 * c : KJ * jj + 8 * g + 4 * (c + 1),
                                    :,
                                ].rearrange("p a b -> p (a b)"),
                                start=True,
                                stop=True,
                                tile_position=(0, 2 * g * RW),
                            )
                            nc.tensor.matmul(
                                pz[
                                    (2 * g + 1) * RW : (2 * g + 2) * RW,
                                    512 * c : 512 * (c + 1),
                                ],
                                onesW[:],
                                z2T[:, 8 * g + 4 * c : 8 * g + 4 * (c + 1), :].rearrange(
                                    "p a b -> p (a b)"
                                ),
                                start=True,
                                stop=True,
                                tile_position=(0, (2 * g + 1) * RW),
                            )
                    # one [112, 1024] bf16 evac per j (rows 112-127 unused);
                    # alternate DVE / ACT (Copy is in every ACT table set)
                    if j % 2 == 0:
                        nc.vector.tensor_copy(
                            ez[:, jj * 1024 : (jj + 1) * 1024], pz[0:112, :]
                        )
                    else:
                        nc.scalar.activation(
                            ez[:, jj * 1024 : (jj + 1) * 1024], pz[0:112, :], AF.Copy
                        )
                    if jj == 0:
                        continue
                    # transpose back to q-partition layout: [128, 16, 112]
                    tT = ztt.tile([128, 16, 112], BF16, tag="tT")
                    nc.sync.dma_start(out=tT, in_=ez[:], transpose=True)
                    # tT[q, b=8*jj+kpl, r]: P0c at r=64*rg+h, S1 at 64*rg+16,
                    # S2 at 64*rg+32, where kp = 16*jj + 8*rg + kpl
                    # (verifier limits APs to 2 free dims -> split per jj)
                    x1 = co_.tile([128, 2, 16], F32, tag="x1")
                    x2 = co_.tile([128, 2, 16], F32, tag="x2")
                    sdz = co_.tile([128, 2, 16], F32, tag="sdz")
                    rstd = co_.tile([128, 2, 16], F32, tag="rstd")
                    for p in range(2):
                        s1 = _ap(tT, H + p * 8 * 112, [[64, 2], [112, 8]])
                        s2 = _ap(tT, 2 * H + p * 8 * 112, [[64, 2], [112, 8]])
                        nc.vector.tensor_mul(x1[:, p, :], s1, s1)
                        nc.vector.scalar_tensor_tensor(
                            out=x2[:, p, :], in0=x1[:, p, :], scalar=-1.0 / DZ,
                            in1=s2, op0=ALU.mult, op1=ALU.add,
                        )
                        nc.scalar.activation(
                            sdz[:, p, :], x2[:, p, :], AF.Sqrt, scale=1.0 / DZ,
                            bias=epsA[:],
                        )
                        nc.vector.reciprocal(rstd[:, p, :], sdz[:, p, :])
                        # bias = rstd * P0c (gpsimd), per 8-kp row-group
                        jp = j - 1 + p
                        half_b = jp // 32
                        kp0 = jp * KJ - half_b * 512
                        bh = bias_h[half_b]
                        for rg in range(2):
                            dst = _ap(bh, (kp0 + 8 * rg) * H, [[H, 8], [1, H]])
                            srcp = _ap(tT, p * 8 * 112 + rg * 64, [[112, 8], [1, H]])
                            rs_ap = _ap(rstd, p * 16 + rg * 8, [[1, 8], [0, H]])
                            nc.gpsimd.tensor_mul(dst, srcp, rs_ap)

            # ---------------- attention ----------------
            if attn_off:
                nc.vector.memset(out_nat[:], 0.5)
            for h in range(0 if not attn_off else H, H):
                po = 64 * (h % 2)
                pr = h // 2
                attn = atp.tile([128, N], BF16, tag=f"attn{h % 2}", name=f"attn_{h}", bufs=1)
                attnT = atp.tile(
                    [128, 8, 128], BF16, tag=f"attnT{h % 2}", name=f"attnT_{h}", bufs=1
                )
                for half in range(2):
                    c0 = half * 512
                    sc = ps_s.tile([128, 512], F32, tag="sc")
                    nc.tensor.matmul(
                        sc,
                        qT[po : po + 48, pr, :],
                        kT[po : po + 48, pr, c0 : c0 + 512],
                        start=True,
                        stop=False,
                    )
                    nc.tensor.matmul(
                        sc,
                        ident[:],
                        bias_h[half][:, :, h],
                        start=False,
                        stop=True,
                    )
                    # |logits| < 2 for this problem: exp without max-subtract
                    nc.scalar.activation(attn[:, c0 : c0 + 512], sc[:], AF.Exp)
                nc.sync.dma_start(out=attnT[:, :, :], in_=attn[:], transpose=True)
                den = atp.tile([128, 1], F32, tag="den")
                nc.vector.reduce_sum(out=den, in_=attn[:], axis=AX.X)
                rden = atp.tile([128, 2, 1], F32, tag=f"rden{h % 2}", name=f"rden_{h}")
                nc.vector.reciprocal(rden[:, 0, :], den[:])
                psA = ps_z.tile([128, DH], F32, tag="pz")
                for kt in range(8):
                    nc.tensor.matmul(
                        psA,
                        attnT[:, kt, :],
                        v_sb[:, kt, DH * h : DH * h + DH],
                        start=(kt == 0),
                        stop=(kt == 7),
                    )
                nc.vector.scalar_tensor_tensor(
                    out=out_nat[:, DH * h : DH * h + DH],
                    in0=psA[:],
                    scalar=rden[:, 0, :],
                    in1=sig_g[:, DH * h : DH * h + DH],
                    op0=ALU.mult,
                    op1=ALU.mult,
                )

            # ---------------- output projection ----------------
            outT = pp.tile([128, 6, QP], BF16)
            nc.sync.dma_start(out=outT[:, :, :], in_=out_nat[:], transpose=True)
            ow_s = wp.tile([128, 6, DA], BF16, tag="w9")
            nc.gpsimd.dma_start(out=ow_s, in_=ow_d[:].rearrange("(t p) n -> p t n", p=128))
            fin = pp.tile([128, DA], F32)
            for c0, cn in chunks:
                psF = ps_m.tile([128, cn], F32, tag="misc")
                for kt in range(6):
                    nc.tensor.matmul(
                        psF,
                        outT[:, kt, :],
                        ow_s[:, kt, c0 : c0 + cn],
                        start=(kt == 0),
                        stop=(kt == 5),
                    )
                nc.vector.tensor_mul(fin[:, c0 : c0 + cn], psF[:], sig_o[:, c0 : c0 + cn])
            nc.sync.dma_start(out=out_d[:], in_=fin[:])

    nc.compile()
    return nc


_CACHE = {}


def _get_program():
    if "nc" not in _CACHE:
        _CACHE["nc"] = build_program()
    return _CACHE["nc"]


def _pad64(w):
    """[DA, HD] -> [DA, HDP] with head h at columns 64h..64h+48."""
    out = np.zeros((w.shape[0], HDP), np.float32)
    for h in range(H):
        out[:, 64 * h : 64 * h + DH] = w[:, DH * h : DH * (h + 1)]
    return out


def _pad64v(v):
    out = np.zeros((HDP,), np.float32)
    for h in range(H):
        out[64 * h : 64 * h + DH] = v[DH * h : DH * (h + 1)]
    return out


def make_in_maps(inputs):
    """Shard full inputs into 8 per-core input maps (host-side staging:
    dtype casts, head padding, and folding SCALE into the Q weights)."""
    bf = ml_dtypes.bfloat16
    f = lambda k: np.asarray(inputs[k], dtype=np.float32)
    a = f("a")[0]
    s = f("s")[0]
    z = f("z")[0].astype(bf)
    shared = {
        "a": a,
        "s": s,
        "adaln_gw": f("adaln_gw").astype(bf),
        "adaln_bw": f("adaln_bw").astype(bf),
        "adaln_gb": f("adaln_gb").astype(bf),
        "qwp": (_pad64(f("qw") * SCALE)).astype(bf),
        "qbp": (_pad64v(f("qb") * SCALE)).astype(bf),
        "kwp": _pad64(f("kw")).astype(bf),
        "vw": f("vw").astype(bf),
        "gw": f("gw").astype(bf),
        "ow": f("ow").astype(bf),
        "zn_g": f("zn_g"),
        "zp_w": f("zp_w"),
        "sg_w": f("sg_w").astype(bf),
        "sg_b": f("sg_b").astype(bf),
    }
    in_maps = []
    for c in range(NCORES):
        sl = slice(c * QP, (c + 1) * QP)
        m = dict(shared)
        m["a_q"] = np.ascontiguousarray(a[sl])
        m["s_q"] = np.ascontiguousarray(s[sl])
        m["z_q"] = np.ascontiguousarray(z[sl])
        in_maps.append(m)
    return in_maps


def kernel(**inputs) -> np.ndarray:
    from concourse.bass_utils import run_bass_kernel_spmd

    nc = _get_program()
    in_maps = make_in_maps(inputs)
    res = run_bass_kernel_spmd(nc, in_maps, core_ids=list(range(NCORES)), trace=False)
    _CACHE["last_results"] = res
    out = np.concatenate([res.results[c]["out"] for c in range(NCORES)], axis=0)
    return out[None].astype(np.float32)



# revision 8
# speedup vs baseline: 1.1527x; 1.1527x over previous
"""DiffusionAttentionPairBias kernel for Trainium2 (8 NeuronCores, SPMD), v4.

Problem (B=1, N=1024, D_A=768, D_S=384, D_Z=128, H=16, DH=48):
  q_in = sigmoid(LN(s) @ gw_ad + gb_ad) * LN(a) + LN(s) @ bw_ad
  q,k,v,g = projections of q_in;  bias = (LN(z)*zn_g + zn_b) @ zp_w
  attn = softmax(q k^T / sqrt(DH) + bias);  out = sigmoid(g) * (attn v)
  y = sigmoid(s @ sg_w + sg_b) * (out @ ow)

Sharding: pure data-parallel on the query axis (core c owns q rows
[128c, 128c+128)). No collectives; host concatenates.

Structure (driven by TimelineSim cost analysis):
 - All weights bf16 host-side; SCALE and qb folded into 64-stride-padded
   Q/K weights (so Q/K project in natural layout, then one wide xbar
   transpose each into the head-pair layout).
 - z LN mean-correction folds into column-centered projection weights:
     Wc = zn_g*zp_w - colsum(zn_g*zp_w)/DZ
   so bias = rstd * (z @ Wc) exactly (zn_b and the mean term are
   constant along the softmax axis / linear in z respectively).
 - z is DMA'd straight from DRAM through the xbar transpose (ch to
   partitions) - no separate load+transpose, half the DMA bus traffic.
 - Per 16-key tile j (64 iterations): one DVE/ACT square, 4 wide
   matmuls (1024 cols) into a [64, 1024] PSUM tile: PE quadrant 0
   streams zT against [Wc | ones | 0] (P0c rows 0-15, S1 row 16),
   quadrant 1 streams z2T against [ones | 0] (S2 row 32). One [48,1024]
   bf16 evac + one xbar transpose per j returns everything to
   q-partition layout; rstd = 1/sqrt((S2-S1^2/DZ)/DZ+eps) and a single
   multiply writes bias_h.
 - ACT function-table thrash avoided: LN stats for all 19 tiles batched
   into one Sqrt phase, then one Sigmoid phase (square/copy are in
   every ACT table set, so z-loop squares on ACT are free).
 - Scores in PSUM: QK^T matmul + identity-matmul bias fold. Softmax:
   exp without max-subtract (|logits| < 2 here), sum/reciprocal folded
   into the AV epilogue.
"""

import math
import os

import ml_dtypes
import numpy as np

import concourse.bass as bass
import concourse.bacc as bacc
import concourse.mybir as mybir
import concourse.tile as tile
from concourse.masks import make_identity

F32 = mybir.dt.float32
BF16 = mybir.dt.bfloat16
AF = mybir.ActivationFunctionType
ALU = mybir.AluOpType
AX = mybir.AxisListType

N = 1024
DA = 768
DS = 384
DZ = 128
H = 16
DH = 48
HD = 768
HDP = 1024        # padded head dim stride (heads at 64-stride)
QP = 128          # query rows per core
NCORES = 8
SCALE = 1.0 / math.sqrt(DH)
EPS = 1e-5
KJ = 16           # kp rows per z macro-tile
NJ = N // KJ      # 64 z macro-tiles
RW = 32           # PE quadrant row stride


def _flag(name):
    return bool(int(os.environ.get(name, "0")))


def _ap(t, offset_elems, dims):
    """Build an AP on tile `t`: partition dim is inherited from the tile,
    `dims` are explicit free dims [[stride, count], ...]."""
    a0 = t[:]
    return bass.AP(
        tensor=a0.tensor,
        offset=a0.offset + offset_elems,
        ap=[list(a0.ap[0])] + [list(d) for d in dims],
    )


# ---- packed-input layouts (flat 1-D buffers; offsets in elements) ----
# wpack (bf16, replicated): adaln_gw, adaln_bw, adaln_gb, qwp, qbp, kwp,
#   vw, gw, ow, sg_w, sg_b
_W_SIZES = [
    ("adaln_gw", DS * DA),
    ("adaln_bw", DS * DA),
    ("adaln_gb", DA),
    ("qwp", DA * HDP),
    ("qbp", HDP),
    ("kwp", DA * HDP),
    ("vw", DA * HD),
    ("gw", DA * HD),
    ("ow", HD * DA),
    ("sg_w", DS * DA),
    ("sg_b", DA),
]
# fpack (f32, per-core): a, s, a_q, s_q, zn_g, zp_w
_F_SIZES = [
    ("a", N * DA),
    ("s", N * DS),
    ("a_q", QP * DA),
    ("s_q", QP * DS),
    ("zn_g", DZ),
    ("zp_w", DZ * H),
]


def _offsets(sizes):
    off, out = 0, {}
    for name, sz in sizes:
        out[name] = (off, sz)
        off += sz
    return out, off


_W_OFF, _W_TOTAL = _offsets(_W_SIZES)
_F_OFF, _F_TOTAL = _offsets(_F_SIZES)


def build_program():
    nc = bacc.Bacc("TRN2", target_bir_lowering=False, debug=False)

    wpack_d = nc.dram_tensor("wpack", [_W_TOTAL], BF16, kind="ExternalInput")
    fpack_d = nc.dram_tensor("fpack", [_F_TOTAL], F32, kind="ExternalInput")
    z_d = nc.dram_tensor("z_q", [QP, N, DZ], BF16, kind="ExternalInput")
    out_d = nc.dram_tensor("out", [QP, DA], F32, kind="ExternalOutput")

    def wsl(name):
        off, sz = _W_OFF[name]
        return wpack_d[off : off + sz]

    def fsl(name):
        off, sz = _F_OFF[name]
        return fpack_d[off : off + sz]

    def frows(name, r0, nrows, ncols):
        off, _ = _F_OFF[name]
        return fpack_d[
            off + r0 * ncols : off + (r0 + nrows) * ncols
        ].rearrange("(p n) -> p n", n=ncols)

    with tile.TileContext(nc) as tc:
        with (
            tc.tile_pool(name="const", bufs=1) as cp,
            tc.tile_pool(name="persist", bufs=1) as pp,
            tc.tile_pool(name="wpool", bufs=2) as wp,
            tc.tile_pool(name="act", bufs=2) as ap_,
            tc.tile_pool(name="ztr", bufs=2) as zt_,
            tc.tile_pool(name="zsq", bufs=4) as zq_,
            tc.tile_pool(name="zev", bufs=2) as ze_,
            tc.tile_pool(name="ztt", bufs=2) as ztt,
            tc.tile_pool(name="corr", bufs=2) as co_,
            tc.tile_pool(name="attnp", bufs=2) as atp,
            tc.tile_pool(name="ps_misc", bufs=1, space="PSUM") as ps_m,
            tc.tile_pool(name="ps_z", bufs=3, space="PSUM") as ps_z,
            tc.tile_pool(name="ps_s", bufs=1, space="PSUM") as ps_s,
        ):
            # ---------------- constants / small prep ----------------
            ones_r = cp.tile([1, 128], BF16)
            nc.vector.memset(ones_r, 1.0)
            epsA = cp.tile([128, 1], F32)
            nc.vector.memset(epsA, EPS)
            ident = cp.tile([128, 128], BF16)
            make_identity(nc, ident)
            ones1 = cp.tile([DZ, 1], BF16)
            nc.vector.memset(ones1, 1.0)

            # waugC = [zn_g*zp_w - colsum/DZ | ones | 0-pad]  ([DZ, RW])
            # onesW = [ones | 0-pad]                          ([DZ, RW])
            zng_c = cp.tile([DZ, 1], F32)
            nc.sync.dma_start(out=zng_c, in_=fsl("zn_g").rearrange("(p o) -> p o", o=1))
            zpw_f = cp.tile([DZ, H], F32)
            nc.sync.dma_start(out=zpw_f, in_=fsl("zp_w").rearrange("(p n) -> p n", n=H))
            wprod = cp.tile([DZ, H], F32)
            nc.vector.tensor_scalar_mul(wprod, in0=zpw_f[:], scalar1=zng_c[:])
            wprod_b = cp.tile([DZ, H], BF16)
            nc.vector.tensor_copy(wprod_b, wprod[:])
            ps_cs = ps_m.tile([1, H], F32, tag="misc")
            nc.tensor.matmul(ps_cs, ones1[:], wprod_b[:], start=True, stop=True)
            cs_row = cp.tile([1, H], F32)
            nc.scalar.activation(cs_row, ps_cs[0:1, :], AF.Copy, scale=1.0 / DZ)
            cs_bf = cp.tile([1, H], BF16)
            nc.vector.tensor_copy(cs_bf, cs_row[:])
            ps_csb = ps_m.tile([128, H], F32, tag="misc")
            nc.tensor.matmul(ps_csb, ones_r[:], cs_bf[:], start=True, stop=True)
            waugC = cp.tile([DZ, RW], BF16)
            nc.vector.memset(waugC, 0.0)
            nc.vector.tensor_sub(waugC[:, 0:H], wprod[:], ps_csb[:])
            nc.vector.memset(waugC[:, H : H + 1], 1.0)
            onesW = cp.tile([DZ, RW], BF16)
            nc.vector.memset(onesW, 0.0)
            nc.vector.memset(onesW[:, 0:1], 1.0)

            # per-feature bias rows (bf16, added via K=1 rank-1 matmuls)
            gbad_r = cp.tile([1, DA], BF16)
            nc.gpsimd.dma_start(
                out=gbad_r, in_=wsl("adaln_gb").rearrange("(o n) -> o n", o=1)
            )
            qbp_r = cp.tile([1, HDP], BF16)
            nc.gpsimd.dma_start(
                out=qbp_r, in_=wsl("qbp").rearrange("(o n) -> o n", o=1)
            )
            sgb_r = cp.tile([1, DA], BF16)
            nc.gpsimd.dma_start(
                out=sgb_r, in_=wsl("sg_b").rearrange("(o n) -> o n", o=1)
            )

            # ---------------- persistent activations ----------------
            s_lnT = pp.tile([128, 3, N], BF16)
            q_inT = pp.tile([128, 6, N], BF16)
            qi_qT = pp.tile([128, 6, QP], BF16)
            kT = pp.tile([128, 8, N], BF16)          # K^T head-pairs at part 0/64
            qT = pp.tile([128, 8, QP], BF16)         # (Q*SCALE+qb)^T head-pairs
            v_sb = pp.tile([128, 8, HD], BF16)
            sig_g = pp.tile([128, HD], BF16)
            sig_o = pp.tile([128, DA], BF16)
            out_nat = pp.tile([128, HD], BF16)
            bias_h = [
                pp.tile([128, 512, H], BF16, tag=f"bias{i}", name=f"bias{i}")
                for i in range(2)
            ]

            z_off = _flag("Z_OFF")
            attn_off = _flag("ATTN_OFF")

            # ---- z prefetch: stream z in ALREADY TRANSPOSED (DRAM xbar),
            # two j-tiles per DMA ----
            PREF = 2
            zld_tiles = {}

            def z_load(m):
                t = zt_.tile([128, 2 * KJ, DZ], BF16, tag="zT")
                nc.sync.dma_start(
                    out=t,
                    in_=z_d[:, 2 * KJ * m : 2 * KJ * (m + 1), :].rearrange(
                        "p a b -> p (a b)"
                    ),
                    transpose=True,
                )
                zld_tiles[m] = t

            if not z_off:
                for m in range(PREF):
                    z_load(m)

            # ---------------- front ----------------
            # s tiles: single-pass LN (Sqrt table stays resident; no reload)
            def ln_full(src_ap, out_bf):
                xt = ap_.tile([128, DS], F32, tag="lnins")
                nc.gpsimd.dma_start(out=xt, in_=src_ap)
                st6 = ap_.tile([128, 2, 6], F32, tag="lnst")
                nc.vector.bn_stats(out=st6[:, 0, :], in_=xt[:, 0 : DS // 2])
                nc.vector.bn_stats(out=st6[:, 1, :], in_=xt[:, DS // 2 : DS])
                mv = ap_.tile([128, 2], F32, tag="lnmv")
                nc.vector.bn_aggr(out=mv, in_=st6[:, :, :])
                sd = ap_.tile([128, 1], F32, tag="lnsd")
                nc.scalar.activation(sd, mv[:, 1:2], AF.Sqrt, bias=epsA[:])
                rs = ap_.tile([128, 1], F32, tag="lnrs")
                nc.vector.reciprocal(rs, sd[:])
                nc.vector.tensor_scalar(
                    out=out_bf, in0=xt[:], scalar1=mv[:, 0:1], scalar2=rs[:],
                    op0=ALU.subtract, op1=ALU.mult,
                )

            s_srcs = [frows("s", t * 128, 128, DS) for t in range(8)] + [
                frows("s_q", 0, QP, DS)
            ]
            a_srcs = [frows("a", t * 128, 128, DA) for t in range(8)] + [
                frows("a_q", 0, QP, DA)
            ]
            for t in range(8):
                s_ln = ap_.tile([128, DS], BF16, tag="sln")
                ln_full(s_srcs[t], s_ln[:])
                nc.sync.dma_start(
                    out=s_lnT[:, :, t * 128 : (t + 1) * 128], in_=s_ln[:], transpose=True
                )
            sq_ln = pp.tile([128, DS], BF16)
            ln_full(frows("s_q", 0, QP, DS), sq_ln[:])
            sq_lnT = pp.tile([128, 3, QP], BF16)
            nc.sync.dma_start(out=sq_lnT[:, :, :], in_=sq_ln[:], transpose=True)

            # a tiles: stats pass now, normalize inside adaln (reload) so the
            # sigmoid phase never interleaves with ACT Sqrt ops
            mv_all = pp.tile([128, 9, 2], F32)
            rs_all = pp.tile([128, 9], F32)
            for t in range(9):
                xt = ap_.tile([128, DA], F32, tag="lnina")
                nc.gpsimd.dma_start(out=xt, in_=a_srcs[t])
                st6 = ap_.tile([128, 2, 6], F32, tag="lnst")
                nc.vector.bn_stats(out=st6[:, 0, :], in_=xt[:, 0 : DA // 2])
                nc.vector.bn_stats(out=st6[:, 1, :], in_=xt[:, DA // 2 : DA])
                nc.vector.bn_aggr(out=mv_all[:, t, :], in_=st6[:, :, :])
            sd_all = pp.tile([128, 9], F32)
            nc.scalar.activation(sd_all, _ap(mv_all, 1, [[2, 9]]), AF.Sqrt, bias=epsA[:])
            nc.vector.reciprocal(rs_all, sd_all[:])

            def ln_norm(t, src_ap, out_bf):
                xt = ap_.tile([128, DA], F32, tag="lnina")
                nc.gpsimd.dma_start(out=xt, in_=src_ap)
                nc.vector.tensor_scalar(
                    out=out_bf, in0=xt[:], scalar1=mv_all[:, t, 0:1],
                    scalar2=rs_all[:, t : t + 1], op0=ALU.subtract, op1=ALU.mult,
                )

            # ---- phase B2: adaln -> q_in (sigmoid only on ACT) ----
            gwad_s = wp.tile([128, 3, DA], BF16, tag="w9")
            nc.gpsimd.dma_start(out=gwad_s, in_=wsl("adaln_gw").rearrange("(t p n) -> p t n", p=128, n=DA))
            bwad_s = wp.tile([128, 3, DA], BF16, tag="w9")
            nc.gpsimd.dma_start(out=bwad_s, in_=wsl("adaln_bw").rearrange("(t p n) -> p t n", p=128, n=DA))

            chunks = [(0, 512), (512, 256)]

            def adaln_qin(t, lnT_ap, a_src, out_T, outT_col0):
                a_ln = ap_.tile([128, DA], BF16, tag="aln")
                ln_norm(t, a_src, a_ln[:])
                q_in = ap_.tile([128, DA], BF16, tag="qin")
                for c0, cn in chunks:
                    psG = ps_m.tile([128, cn], F32, tag="misc")
                    for kt in range(3):
                        nc.tensor.matmul(
                            psG,
                            lnT_ap(kt),
                            gwad_s[:, kt, c0 : c0 + cn],
                            start=(kt == 0),
                            stop=False,
                        )
                    nc.tensor.matmul(
                        psG, ones_r[:], gbad_r[:, c0 : c0 + cn], start=False, stop=True
                    )
                    sgG = ap_.tile([128, cn], BF16, tag="sgG")
                    nc.scalar.activation(sgG, psG[:], AF.Sigmoid)
                    psB = ps_m.tile([128, cn], F32, tag="misc")
                    for kt in range(3):
                        nc.tensor.matmul(
                            psB,
                            lnT_ap(kt),
                            bwad_s[:, kt, c0 : c0 + cn],
                            start=(kt == 0),
                            stop=(kt == 2),
                        )
                    tmp = ap_.tile([128, cn], BF16, tag="qtmp")
                    nc.vector.tensor_mul(tmp, sgG[:], a_ln[:, c0 : c0 + cn])
                    nc.vector.tensor_add(q_in[:, c0 : c0 + cn], tmp[:], psB[:])
                nc.sync.dma_start(
                    out=out_T[:, :, outT_col0 : outT_col0 + 128], in_=q_in[:], transpose=True
                )

            for t in range(8):
                adaln_qin(
                    t,
                    lambda kt, t=t: s_lnT[:, kt, t * 128 : (t + 1) * 128],
                    a_srcs[t],
                    q_inT,
                    t * 128,
                )
            adaln_qin(8, lambda kt: sq_lnT[:, kt, :], frows("a_q", 0, QP, DA), qi_qT, 0)

            # ---- K natural through padded weights, then wide transposes ----
            kwp_s = wp.tile([128, 6, HDP], BF16, tag="w9")
            nc.gpsimd.dma_start(out=kwp_s, in_=wsl("kwp").rearrange("(t p n) -> p t n", p=128, n=HDP))
            for t in range(8):
                k_nat = ap_.tile([128, HDP], BF16, tag="knat")
                for c0 in (0, 512):
                    psK = ps_m.tile([128, 512], F32, tag="misc")
                    for kt in range(6):
                        nc.tensor.matmul(
                            psK,
                            q_inT[:, kt, t * 128 : (t + 1) * 128],
                            kwp_s[:, kt, c0 : c0 + 512],
                            start=(kt == 0),
                            stop=(kt == 5),
                        )
                    nc.vector.tensor_copy(k_nat[:, c0 : c0 + 512], psK[:])
                nc.sync.dma_start(
                    out=kT[:, :, t * 128 : (t + 1) * 128], in_=k_nat[:], transpose=True
                )

            # ---- V natural ----
            vw_s = wp.tile([128, 6, HD], BF16, tag="w9")
            nc.gpsimd.dma_start(out=vw_s, in_=wsl("vw").rearrange("(t p n) -> p t n", p=128, n=HD))
            for t in range(8):
                for c0, cn in chunks:
                    psV = ps_m.tile([128, 512], F32, tag="misc")
                    for kt in range(6):
                        nc.tensor.matmul(
                            psV[:, 0:cn],
                            q_inT[:, kt, t * 128 : (t + 1) * 128],
                            vw_s[:, kt, c0 : c0 + cn],
                            start=(kt == 0),
                            stop=(kt == 5),
                        )
                    nc.vector.tensor_copy(v_sb[:, t, c0 : c0 + cn], psV[:, 0:cn])

            # ---- Q natural through padded+scaled weights ----
            qwp_s = wp.tile([128, 6, HDP], BF16, tag="w9")
            nc.gpsimd.dma_start(out=qwp_s, in_=wsl("qwp").rearrange("(t p n) -> p t n", p=128, n=HDP))
            q_nat = ap_.tile([128, HDP], BF16, tag="knat")
            for c0 in (0, 512):
                psQ = ps_m.tile([128, 512], F32, tag="misc")
                for kt in range(6):
                    nc.tensor.matmul(
                        psQ,
                        qi_qT[:, kt, :],
                        qwp_s[:, kt, c0 : c0 + 512],
                        start=(kt == 0),
                        stop=False,
                    )
                nc.tensor.matmul(
                    psQ, ones_r[:], qbp_r[:, c0 : c0 + 512], start=False, stop=True
                )
                nc.vector.tensor_copy(q_nat[:, c0 : c0 + 512], psQ[:])
            nc.sync.dma_start(out=qT[:, :, :], in_=q_nat[:], transpose=True)

            # ---- G gate ----
            gw_s = wp.tile([128, 6, HD], BF16, tag="w9")
            nc.gpsimd.dma_start(out=gw_s, in_=wsl("gw").rearrange("(t p n) -> p t n", p=128, n=HD))
            for c0, cn in chunks:
                psg = ps_m.tile([128, cn], F32, tag="misc")
                for kt in range(6):
                    nc.tensor.matmul(
                        psg,
                        qi_qT[:, kt, :],
                        gw_s[:, kt, c0 : c0 + cn],
                        start=(kt == 0),
                        stop=(kt == 5),
                    )
                nc.scalar.activation(sig_g[:, c0 : c0 + cn], psg[:], AF.Sigmoid)

            # ---- output gate from raw s_q ----
            sgw_s = wp.tile([128, 3, DA], BF16, tag="w9")
            nc.gpsimd.dma_start(out=sgw_s, in_=wsl("sg_w").rearrange("(t p n) -> p t n", p=128, n=DA))
            sq_bf = ap_.tile([128, DS], BF16, tag="sqbf")
            nc.gpsimd.dma_start(out=sq_bf, in_=frows("s_q", 0, QP, DS))
            sqT = pp.tile([128, 3, QP], BF16)
            nc.sync.dma_start(out=sqT[:, :, :], in_=sq_bf[:], transpose=True)
            for c0, cn in chunks:
                pso = ps_m.tile([128, cn], F32, tag="misc")
                for kt in range(3):
                    nc.tensor.matmul(
                        pso,
                        sqT[:, kt, :],
                        sgw_s[:, kt, c0 : c0 + cn],
                        start=(kt == 0),
                        stop=False,
                    )
                nc.tensor.matmul(
                    pso, ones_r[:], sgb_r[:, c0 : c0 + cn], start=False, stop=True
                )
                nc.scalar.activation(sig_o[:, c0 : c0 + cn], pso[:], AF.Sigmoid)

            # ---------------- z pipeline ----------------
            if z_off:
                nc.vector.memset(bias_h[0][:], 0.0)
                nc.vector.memset(bias_h[1][:], 0.0)
            else:
                ez = None
                for j in range(NJ):
                    m, jj = j // 2, j % 2
                    if jj == 0:
                        if m + PREF < NJ // 2:
                            z_load(m + PREF)
                        ez = ze_.tile([112, 2 * 8 * DZ], BF16, tag="ez")
                    zT2 = zld_tiles[m]
                    # square (for sum-of-squares); ACT every 4th (square is
                    # in every ACT table set -> no table reload)
                    z2T = zq_.tile([128, KJ, DZ], BF16, tag="z2")
                    zTj = zT2[:, KJ * jj : KJ * (jj + 1), :]
                    if j % 4 == 3:
                        nc.scalar.activation(z2T, zTj, AF.Square)
                    else:
                        nc.vector.tensor_mul(z2T, zTj, zTj)
                    if jj == 1:
                        zld_tiles.pop(m)
                    # one [128, 1024] psum per j, all 4 PE quadrants:
                    #  q0: P0c+S1 kp 0-8 (zT vs waugC)   q1: S2 kp 0-8 (z2T vs onesW)
                    #  q2: P0c+S1 kp 8-16                q3: S2 kp 8-16
                    pz = ps_z.tile([128, 8 * DZ], F32, tag="pz")
                    for g in range(2):
                        for c in range(2):
                            nc.tensor.matmul(
                                pz[2 * g * RW : (2 * g + 1) * RW, 512 * c : 512 * (c + 1)],
                                waugC[:],
                                zT2[
                                    :,
                                    KJ * jj + 8 * g + 4 * c : KJ * jj + 8 * g + 4 * (c + 1),
                                    :,
                                ].rearrange("p a b -> p (a b)"),
                                start=True,
                                stop=True,
                                tile_position=(0, 2 * g * RW),
                            )
                            nc.tensor.matmul(
                                pz[
                                    (2 * g + 1) * RW : (2 * g + 2) * RW,
                                    512 * c : 512 * (c + 1),
                                ],
                                onesW[:],
                                z2T[:, 8 * g + 4 * c : 8 * g + 4 * (c + 1), :].rearrange(
                                    "p a b -> p (a b)"
                                ),
                                start=True,
                                stop=True,
                                tile_position=(0, (2 * g + 1) * RW),
                            )
                    # one [112, 1024] bf16 evac per j (rows 112-127 unused);
                    # alternate DVE / ACT (Copy is in every ACT table set)
                    if j % 2 == 0:
                        nc.vector.tensor_copy(
                            ez[:, jj * 1024 : (jj + 1) * 1024], pz[0:112, :]
                        )
                    else:
                        nc.scalar.activation(
                            ez[:, jj * 1024 : (jj + 1) * 1024], pz[0:112, :], AF.Copy
                        )
                    if jj == 0:
                        continue
                    # transpose back to q-partition layout: [128, 16, 112]
                    tT = ztt.tile([128, 16, 112], BF16, tag="tT")
                    nc.sync.dma_start(out=tT, in_=ez[:], transpose=True)
                    # tT[q, b=8*jj+kpl, r]: P0c at r=64*rg+h, S1 at 64*rg+16,
                    # S2 at 64*rg+32, where kp = 16*jj + 8*rg + kpl
                    # (verifier limits APs to 2 free dims -> split per jj)
                    x1 = co_.tile([128, 2, 16], F32, tag="x1")
                    x2 = co_.tile([128, 2, 16], F32, tag="x2")
                    sdz = co_.tile([128, 2, 16], F32, tag="sdz")
                    rstd = co_.tile([128, 2, 16], F32, tag="rstd")
                    for p in range(2):
                        s1 = _ap(tT, H + p * 8 * 112, [[64, 2], [112, 8]])
                        s2 = _ap(tT, 2 * H + p * 8 * 112, [[64, 2], [112, 8]])
                        nc.vector.tensor_mul(x1[:, p, :], s1, s1)
                        nc.vector.scalar_tensor_tensor(
                            out=x2[:, p, :], in0=x1[:, p, :], scalar=-1.0 / DZ,
                            in1=s2, op0=ALU.mult, op1=ALU.add,
                        )
                        nc.scalar.activation(
                            sdz[:, p, :], x2[:, p, :], AF.Sqrt, scale=1.0 / DZ,
                            bias=epsA[:],
                        )
                        nc.vector.reciprocal(rstd[:, p, :], sdz[:, p, :])
                        # bias = rstd * P0c (gpsimd), per 8-kp row-group
                        jp = j - 1 + p
                        half_b = jp // 32
                        kp0 = jp * KJ - half_b * 512
                        bh = bias_h[half_b]
                        for rg in range(2):
                            dst = _ap(bh, (kp0 + 8 * rg) * H, [[H, 8], [1, H]])
                            srcp = _ap(tT, p * 8 * 112 + rg * 64, [[112, 8], [1, H]])
                            rs_ap = _ap(rstd, p * 16 + rg * 8, [[1, 8], [0, H]])
                            nc.gpsimd.tensor_mul(dst, srcp, rs_ap)

            # ---------------- attention ----------------
            if attn_off:
                nc.vector.memset(out_nat[:], 0.5)
            for h in range(0 if not attn_off else H, H):
                po = 64 * (h % 2)
                pr = h // 2
                attn = atp.tile([128, N], BF16, tag=f"attn{h % 2}", name=f"attn_{h}", bufs=1)
                attnT = atp.tile(
                    [128, 8, 128], BF16, tag=f"attnT{h % 2}", name=f"attnT_{h}", bufs=1
                )
                for half in range(2):
                    c0 = half * 512
                    sc = ps_s.tile([128, 512], F32, tag="sc")
                    nc.tensor.matmul(
                        sc,
                        qT[po : po + 48, pr, :],
                        kT[po : po + 48, pr, c0 : c0 + 512],
                        start=True,
                        stop=False,
                    )
                    nc.tensor.matmul(
                        sc,
                        ident[:],
                        bias_h[half][:, :, h],
                        start=False,
                        stop=True,
                    )
                    # |logits| < 2 for this problem: exp without max-subtract
                    nc.scalar.activation(attn[:, c0 : c0 + 512], sc[:], AF.Exp)
                nc.sync.dma_start(out=attnT[:, :, :], in_=attn[:], transpose=True)
                den = atp.tile([128, 1], F32, tag="den")
                nc.vector.reduce_sum(out=den, in_=attn[:], axis=AX.X)
                rden = atp.tile([128, 2, 1], F32, tag=f"rden{h % 2}", name=f"rden_{h}")
                nc.vector.reciprocal(rden[:, 0, :], den[:])
                psA = ps_z.tile([128, DH], F32, tag="pz")
                for kt in range(8):
                    nc.tensor.matmul(
                        psA,
                        attnT[:, kt, :],
                        v_sb[:, kt, DH * h : DH * h + DH],
                        start=(kt == 0),
                        stop=(kt == 7),
                    )
                nc.vector.scalar_tensor_tensor(
                    out=out_nat[:, DH * h : DH * h + DH],
                    in0=psA[:],
                    scalar=rden[:, 0, :],
                    in1=sig_g[:, DH * h : DH * h + DH],
                    op0=ALU.mult,
                    op1=ALU.mult,
                )

            # ---------------- output projection ----------------
            outT = pp.tile([128, 6, QP], BF16)
            nc.sync.dma_start(out=outT[:, :, :], in_=out_nat[:], transpose=True)
            ow_s = wp.tile([128, 6, DA], BF16, tag="w9")
            nc.gpsimd.dma_start(out=ow_s, in_=wsl("ow").rearrange("(t p n) -> p t n", p=128, n=DA))
            fin = pp.tile([128, DA], F32)
            for c0, cn in chunks:
                psF = ps_m.tile([128, cn], F32, tag="misc")
                for kt in range(6):
                    nc.tensor.matmul(
                        psF,
                        outT[:, kt, :],
                        ow_s[:, kt, c0 : c0 + cn],
                        start=(kt == 0),
                        stop=(kt == 5),
                    )
                nc.vector.tensor_mul(fin[:, c0 : c0 + cn], psF[:], sig_o[:, c0 : c0 + cn])
            nc.sync.dma_start(out=out_d[:], in_=fin[:])

    nc.compile()
    return nc


_CACHE = {}


def _get_program():
    if "nc" not in _CACHE:
        _CACHE["nc"] = build_program()
    return _CACHE["nc"]


def _pad64(w):
    """[DA, HD] -> [DA, HDP] with head h at columns 64h..64h+48."""
    out = np.zeros((w.shape[0], HDP), np.float32)
    for h in range(H):
        out[:, 64 * h : 64 * h + DH] = w[:, DH * h : DH * (h + 1)]
    return out


def _pad64v(v):
    out = np.zeros((HDP,), np.float32)
    for h in range(H):
        out[64 * h : 64 * h + DH] = v[DH * h : DH * (h + 1)]
    return out


def make_in_maps(inputs):
    """Shard full inputs into 8 per-core input maps (host-side staging:
    dtype casts, head padding, folding SCALE into the Q weights, and
    packing everything into 3 flat buffers to minimize dispatch cost)."""
    bf = ml_dtypes.bfloat16
    f = lambda k: np.asarray(inputs[k], dtype=np.float32)
    a = f("a")[0]
    s = f("s")[0]
    z = f("z")[0].astype(bf)
    wparts = {
        "adaln_gw": f("adaln_gw"),
        "adaln_bw": f("adaln_bw"),
        "adaln_gb": f("adaln_gb"),
        "qwp": _pad64(f("qw") * SCALE),
        "qbp": _pad64v(f("qb") * SCALE),
        "kwp": _pad64(f("kw")),
        "vw": f("vw"),
        "gw": f("gw"),
        "ow": f("ow"),
        "sg_w": f("sg_w"),
        "sg_b": f("sg_b"),
    }
    wpack = np.empty((_W_TOTAL,), bf)
    for name, sz in _W_SIZES:
        off, _ = _W_OFF[name]
        wpack[off : off + sz] = wparts[name].astype(bf).ravel()
    in_maps = []
    for c in range(NCORES):
        sl = slice(c * QP, (c + 1) * QP)
        fparts = {
            "a": a,
            "s": s,
            "a_q": a[sl],
            "s_q": s[sl],
            "zn_g": f("zn_g"),
            "zp_w": f("zp_w"),
        }
        fpack = np.empty((_F_TOTAL,), np.float32)
        for name, sz in _F_SIZES:
            off, _ = _F_OFF[name]
            fpack[off : off + sz] = fparts[name].ravel()
        in_maps.append(
            {"wpack": wpack, "fpack": fpack, "z_q": np.ascontiguousarray(z[sl])}
        )
    return in_maps


def kernel(**inputs) -> np.ndarray:
    from concourse.bass_utils import run_bass_kernel_spmd

    nc = _get_program()
    in_maps = make_in_maps(inputs)
    res = run_bass_kernel_spmd(nc, in_maps, core_ids=list(range(NCORES)), trace=False)
    _CACHE["last_results"] = res
    out = np.concatenate([res.results[c]["out"] for c in range(NCORES)], axis=0)
    return out[None].astype(np.float32)

